# revision 14
# baseline (speedup 1.0000x reference)
"""Bidirectional H=1 LSTM attention kernel for Trainium2 (8 NeuronCores).

Model: hs = BiLSTM(x) [B,T,2] -> att = softmax(mean(hs,-1), axis=T) -> out = att[:,:,None]*x
Shapes: B=32, T=4096, E=300, H=1.

v4: bf16 datapath + 2-batch-group software pipeline.
  - Partition layout p = g*64 + d*32 + brel*16 + k (g = batch group of 2,
    d = direction, brel = batch within group, k = chunk).  Each group's scan
    touches one contiguous 64-row range, so group pipelining is legal for
    the compute engines (partition starts 0/32/64/96).
  - Emission interleaves: g0-p1 | g1-p1 + g0-scan | g0-p5 + g1-scan | g1-p5
    at per-block/per-iteration granularity, so the in-order engines overlap
    phases (PE crunches g1 matmuls while DVE/ACT scan g0, output DMA of g0
    runs under g1's scan).
  - fwd/bwd row pairing for the softmax is restored by a 32x32 block-reversal
    permutation matmul on the PE (DMA gathers keep ascending strides).
  - att broadcast to 128 partitions via K=1 PE outer product into PSUM.
"""

import sys

sys.path.insert(0, "/opt/trn_rl_repo")

import numpy as np
from contextlib import ExitStack

import concourse.bass as bass
import concourse.bacc as bacc
import concourse.tile as tile
from concourse import mybir
from concourse.bass_utils import run_bass_kernel_spmd

F32 = mybir.dt.float32
USE_BF16 = True
F16 = mybir.dt.bfloat16 if USE_BF16 else mybir.dt.float16
import ml_dtypes
NP16 = ml_dtypes.bfloat16 if USE_BF16 else np.float16
AF = mybir.ActivationFunctionType
ALU = mybir.AluOpType

NCORES = 8
B, T, E = 32, 4096, 300
BL = B // NCORES          # batches per core
TOK = BL * T              # tokens per core (b-major)
L, W = 256, 32            # chunk len, halo warmup
S = L + W                 # scan steps per chunk
K = T // L                # chunks per (dir, batch)
P = 128                   # partitions = g*64 + d*32 + brel*16 + k
NG = 2                    # batch groups (pipeline stages)
GTOK = TOK // NG          # tokens per group
N_ITER = 4                # fixed-point iterations (validated offline)
PADROW = W + T + W        # padded xg row per batch
CB = 2048                 # token block for loads / p5
NBG = GTOK // CB          # blocks per group (4)
# gate order inside a block row: (i, f, o, g) ; pytorch order is (i, f, g, o)
GATE_PERM = [0, 1, 3, 2]


def _build_nc():
    nc = bacc.Bacc(None, target_bir_lowering=False, debug=False)
    xT = nc.declare_dram_parameter("xT", [E, TOK], F16, isOutput=False)
    w8T = nc.declare_dram_parameter("w8T", [E, 36], F16, isOutput=False)
    b8 = nc.declare_dram_parameter("b8", [36, 1], F32, isOutput=False)
    whh = nc.declare_dram_parameter("whh", [P, 4], F32, isOutput=False)
    selG = nc.declare_dram_parameter("selG", [32, 2], F32, isOutput=False)
    selGT = nc.declare_dram_parameter("selGT", [2, 32], F32, isOutput=False)
    permG = nc.declare_dram_parameter("permG", [32, 32], F16, isOutput=False)
    outT = nc.declare_dram_parameter("outT", [E, TOK], F16, isOutput=True)

    # internal DRAM scratch: rows b*8 + d*4 + g
    dxg = nc.dram_tensor("dxg", [32, PADROW], F16)
    datt = nc.dram_tensor("datt", [1, TOK], F16)   # att, flat token order

    with tile.TileContext(nc) as tc, ExitStack() as ctx:
        singles = ctx.enter_context(tc.tile_pool(name="singles", bufs=1))
        stage = ctx.enter_context(tc.tile_pool(name="stage", bufs=2))
        scanp = ctx.enter_context(tc.tile_pool(name="scanp", bufs=1))
        papool = ctx.enter_context(tc.tile_pool(name="papool", bufs=2))
        opool = ctx.enter_context(tc.tile_pool(name="opool", bufs=4))
        o2pool = ctx.enter_context(tc.tile_pool(name="o2pool", bufs=2))
        psA = ctx.enter_context(tc.tile_pool(name="psA", bufs=2, space="PSUM"))
        psB = ctx.enter_context(tc.tile_pool(name="psB", bufs=1, space="PSUM"))

        # ---- constants / resident tiles ----
        w8a = singles.tile([128, 36], F16)
        w8b = singles.tile([128, 36], F16)
        w8c = singles.tile([44, 36], F16)
        nc.gpsimd.dma_start(out=w8a, in_=w8T[0:128, :])
        nc.gpsimd.dma_start(out=w8b, in_=w8T[128:256, :])
        nc.gpsimd.dma_start(out=w8c, in_=w8T[256:300, :])
        b8_sb = singles.tile([36, 1], F32)
        nc.sync.dma_start(out=b8_sb, in_=b8[:, :])
        whh_sb = singles.tile([P, 4], F32)
        nc.sync.dma_start(out=whh_sb, in_=whh[:, :])
        sel_sb = singles.tile([32, 2], F32)
        nc.sync.dma_start(out=sel_sb, in_=selG[:, :])
        selT_sb = singles.tile([2, 32], F32)
        nc.sync.dma_start(out=selT_sb, in_=selGT[:, :])
        perm_sb = singles.tile([32, 32], F16)
        nc.sync.dma_start(out=perm_sb, in_=permG[:, :])

        xT0 = singles.tile([128, TOK], F16)   # e 0..127 resident
        xT1 = singles.tile([128, TOK], F16)   # e 128..255 resident
        xT2 = singles.tile([44, TOK], F16)    # e 256..299 resident

        # zero-pad regions of dxg (halo reads beyond sequence ends)
        zpad = singles.tile([32, W], F16)
        nc.vector.memset(zpad[:, :], 0.0)
        nc.sync.dma_start(out=dxg[:, 0:W], in_=zpad[:, :])
        nc.sync.dma_start(out=dxg[:, W + T:PADROW], in_=zpad[:, :])

        # scan-layout xg: rows p = g*64 + d*32 + brel*16 + k, cols g*S + s
        xg_tile = singles.tile([128, 4 * S], F16)
        h_st = singles.tile([128, S + 1], F16)   # col 0 stays zero
        nc.vector.memset(h_st[:, :], 0.0)

        # ---------------- emission helpers ----------------
        def emit_p1_block(g, i):
            """Load 2048 tokens, matmul xg, evacuate, write dxg, gather."""
            tt = g * NBG + i
            b = (tt * CB) // T
            toff = (tt * CB) % T
            cols = slice(tt * CB, (tt + 1) * CB)
            nc.gpsimd.dma_start(out=xT0[:, cols], in_=xT[0:128, cols])
            nc.sync.dma_start(out=xT1[:, cols], in_=xT[128:256, cols])
            nc.scalar.dma_start(out=xT2[:, cols], in_=xT[256:300, cols])
            st8 = stage.tile([36, CB], F16, tag="st8")
            c0 = tt * CB
            first = True
            for half in range(2):
                ps = psA.tile([36, 1024], F32, tag="ps")
                if first:
                    first = False
                    # touch matmuls: absorb each fresh DMA's semaphore on the
                    # PE clock (Matmult codegen has one sync-wait slot).
                    nc.tensor.matmul(ps[0:2, 0:2], lhsT=xT0[:, c0:c0 + 2],
                                     rhs=xT0[:, c0:c0 + 2], start=True,
                                     stop=True)
                    nc.tensor.matmul(ps[0:2, 2:4], lhsT=xT1[:, c0:c0 + 2],
                                     rhs=xT1[:, c0:c0 + 2], start=True,
                                     stop=True)
                    nc.tensor.matmul(ps[0:2, 4:6], lhsT=xT2[:, c0:c0 + 2],
                                     rhs=xT2[:, c0:c0 + 2], start=True,
                                     stop=True)
                    if tt == 0:
                        nc.tensor.matmul(ps[0:2, 6:8], lhsT=sel_sb[:, 0:2],
                                         rhs=sel_sb[:, 0:2], start=True,
                                         stop=True)
                        nc.tensor.matmul(ps[0:2, 8:10], lhsT=selT_sb[:, 0:2],
                                         rhs=selT_sb[:, 0:2], start=True,
                                         stop=True)
                        nc.tensor.matmul(ps[0:2, 12:14], lhsT=perm_sb[:, 0:2],
                                         rhs=perm_sb[:, 0:2], start=True,
                                         stop=True)
                for ci, (wch, xch) in enumerate(
                        [(w8a, xT0), (w8b, xT1), (w8c, xT2)]):
                    for n in range(2):
                        cs = slice(tt * CB + half * 1024 + n * 512,
                                   tt * CB + half * 1024 + n * 512 + 512)
                        pss = ps[:, n * 512:(n + 1) * 512]
                        nc.tensor.matmul(pss, lhsT=wch, rhs=xch[:, cs],
                                         start=(ci == 0), stop=(ci == 2),
                                         skip_group_check=True)
                # evacuate + bias on ACT (single psA reader keeps matmul WAR
                # to one semaphore)
                nc.scalar.activation(st8[:, half * 1024:(half + 1) * 1024],
                                     ps, AF.Identity, bias=b8_sb[:, :],
                                     scale=1.0)
            dst0 = W + toff
            nc.sync.dma_start(out=dxg[b * 8:b * 8 + 4, dst0:dst0 + CB],
                              in_=st8[0:4, :])
            # d=1 rows stored time-REVERSED (col W+r holds t=T-1-r)
            strev = stage.tile([4, CB], F16, tag="strev")
            nc.vector.tensor_copy(strev, st8[32:36, ::-1])
            lo = PADROW - CB - dst0
            nc.sync.dma_start(out=dxg[b * 8 + 4:b * 8 + 8, lo:lo + CB],
                              in_=strev)
            if toff + CB == T:
                # batch b complete: gather both dirs into scan layout.
                # dst rows g*64 + d*32 + brel*16 + k; src runs of S elems;
                # bwd rows hold scan chunk K-1-khat (ascending strides; the
                # att stage repairs pairing with a PE permutation).
                brel = b % 2
                base = dxg[:, :]
                for d in range(2):
                    src = bass.AP(
                        tensor=base.tensor,
                        offset=(b * 8 + d * 4) * PADROW,
                        ap=[[L, K], [PADROW, 4], [1, S]])
                    r0 = g * 64 + d * 32 + brel * 16
                    nc.sync.dma_start(
                        out=xg_tile[r0:r0 + 16, :].rearrange(
                            "p (g s) -> p g s", g=4),
                        in_=src)

        def emit_scan_iter(g, it):
            r = slice(g * 64, (g + 1) * 64)
            if it == 0:
                gsrc = xg_tile
            else:
                gsrc = scanp.tile([64, 4 * S], F16, tag=f"gbuf{g}")
                for gg in (0, 3, 1, 2):   # i, g first: unblocks mt earliest
                    nc.vector.scalar_tensor_tensor(
                        out=gsrc[:, gg * S:(gg + 1) * S],
                        in0=h_st[r, 0:S],
                        scalar=whh_sb[r, gg:gg + 1],
                        in1=xg_tile[r, gg * S:(gg + 1) * S],
                        op0=ALU.mult, op1=ALU.add)

            def gs(a, b_):
                return (gsrc[r, a:b_] if it == 0 else gsrc[:, a:b_])
            St = scanp.tile([64, 3 * S], F16, tag=f"St{g}")
            Gt = scanp.tile([64, S], F16, tag=f"Gt{g}")
            mt = scanp.tile([64, S], F16, tag=f"mt{g}")
            ct = scanp.tile([64, S], F16, tag=f"ct{g}")
            tct = scanp.tile([64, S], F16, tag=f"tct{g}")
            nc.scalar.activation(St[:, 0:S], gs(0, S), AF.Sigmoid)
            nc.scalar.activation(Gt, gs(3 * S, 4 * S), AF.Tanh)
            nc.vector.tensor_mul(mt, St[:, 0:S], Gt)
            nc.scalar.activation(St[:, S:2 * S], gs(S, 2 * S), AF.Sigmoid)
            nc.vector.tensor_tensor_scan(
                out=ct, data0=St[:, S:2 * S], data1=mt, initial=0.0,
                op0=ALU.mult, op1=ALU.add)
            nc.scalar.activation(St[:, 2 * S:3 * S], gs(2 * S, 3 * S),
                                 AF.Sigmoid)
            nc.scalar.activation(tct, ct, AF.Tanh)
            nc.vector.tensor_mul(h_st[r, 1:S + 1], St[:, 2 * S:3 * S], tct)

        def emit_att(g):
            r0 = g * 64
            h_rev = scanp.tile([32, S + 1], F16, tag=f"hrev{g}")
            nc.vector.tensor_copy(h_rev, h_st[r0 + 32:r0 + 64, ::-1])
            # one PSUM bank holds all three small attention results
            psb = psB.tile([32, 512], F32, tag="psb")
            hb_perm = psb[:, 0:L]
            nc.tensor.matmul(hb_perm, lhsT=perm_sb, rhs=h_rev[:, 0:L],
                             start=True, stop=True)
            hsum = scanp.tile([32, L], F16, tag=f"hsum{g}")
            nc.vector.tensor_add(hsum, h_st[r0:r0 + 32, W + 1:S + 1], hb_perm)
            negone = scanp.tile([32, 1], F32, tag=f"negone{g}")
            nc.vector.memset(negone[:, :], -1.0)
            exps = scanp.tile([32, L], F32, tag=f"exps{g}")
            s1 = scanp.tile([32, 1], F32, tag=f"s1{g}")
            # exp(0.5*hsum - 1) in [e^-2, 1]: stable without max-subtraction
            nc.scalar.activation(exps, hsum, AF.Exp, bias=negone[:, :],
                                 scale=0.5, accum_out=s1)
            ps_s = psb[0:2, 300:301]
            nc.tensor.matmul(ps_s, lhsT=sel_sb, rhs=s1, start=True, stop=True)
            r2 = scanp.tile([2, 1], F32, tag=f"r2{g}")
            nc.vector.reciprocal(r2, ps_s)
            ps_r = psb[:, 320:321]
            nc.tensor.matmul(ps_r, lhsT=selT_sb, rhs=r2, start=True, stop=True)
            att_r = scanp.tile([32, L], F16, tag=f"attr{g}")
            nc.vector.tensor_scalar_mul(att_r, exps, ps_r[:, 0:1])
            # flatten rows into token order: row r=(brel*16+k) -> offset r*L
            # (gpsimd queue: keeps the sync queue free for loads/stores)
            nc.gpsimd.dma_start(
                out=datt[0:1, g * GTOK:(g + 1) * GTOK].rearrange(
                    "p (r s) -> p r s", r=32),
                in_=att_r[:, :])

        def emit_p5_block(g, i):
            tt = g * NBG + i
            cols = slice(tt * CB, (tt + 1) * CB)
            # broadcast att across 128 partitions with a stride-0 SWDGE DMA
            # (gpsimd queue idle in p5; no PE/ACT involvement)
            pa = papool.tile([128, CB], F16, tag="pa")
            nc.gpsimd.dma_start(
                out=pa,
                in_=bass.AP(tensor=datt[:, :].tensor, offset=tt * CB,
                            ap=[[0, 128], [1, CB]]))
            ob0 = opool.tile([128, CB], F16, tag="ob")
            nc.vector.tensor_mul(ob0, xT0[:, cols], pa)
            nc.sync.dma_start(out=outT[0:128, cols], in_=ob0)
            ob1 = opool.tile([128, CB], F16, tag="ob")
            nc.vector.tensor_mul(ob1, xT1[:, cols], pa)
            nc.scalar.dma_start(out=outT[128:256, cols], in_=ob1)
            ob2 = o2pool.tile([44, CB], F16, tag="ob2")
            nc.vector.tensor_mul(ob2, xT2[:, cols], pa[0:44, :])
            nc.scalar.dma_start(out=outT[256:300, cols], in_=ob2)

        # ---------------- pipeline schedule ----------------
        for i in range(NBG):
            emit_p1_block(0, i)
        for i in range(NBG - 1):
            emit_p1_block(1, i)
            emit_scan_iter(0, i)
        emit_scan_iter(0, NBG - 1)
        # att(0) PE ops emitted before g1's last matmul block so the in-order
        # PE stream reaches them as soon as g0's scan finishes
        emit_att(0)
        emit_p1_block(1, NBG - 1)
        for i in range(NBG):
            emit_p5_block(0, i)
            emit_scan_iter(1, i)
        emit_att(1)
        for i in range(NBG):
            emit_p5_block(1, i)

    return nc


_NC = None


def _get_nc():
    global _NC
    if _NC is None:
        _NC = _build_nc()
        _NC.finalize()
    return _NC


def _prep_core_inputs(x, w_ih_f, w_hh_f, b_ih_f, b_hh_f,
                      w_ih_b, w_hh_b, b_ih_b, b_hh_b):
    """Build the per-core input maps."""
    w8T = np.zeros((E, 36), np.float32)
    b8 = np.zeros((36, 1), np.float32)
    whh = np.zeros((P, 4), np.float32)
    for d, (wi, wh, bi, bh) in enumerate(
            [(w_ih_f, w_hh_f, b_ih_f, b_hh_f),
             (w_ih_b, w_hh_b, b_ih_b, b_hh_b)]):
        for j, gp in enumerate(GATE_PERM):
            w8T[:, d * 32 + j] = wi[gp, :]
            b8[d * 32 + j, 0] = bi[gp] + bh[gp]
            # rows p = g*64 + d*32 + brel*16 + k: d = (p//32) % 2
            for gg in range(2):
                whh[gg * 64 + d * 32:gg * 64 + (d + 1) * 32, j] = wh[gp, 0]
    selG = np.zeros((32, 2), np.float32)
    for r in range(32):
        selG[r, r // 16] = 1.0
    selGT = np.ascontiguousarray(selG.T)
    permG = np.zeros((32, 32), NP16)
    for bb in range(2):
        for i in range(16):
            permG[bb * 16 + i, bb * 16 + 15 - i] = 1.0
    w8T16 = w8T.astype(NP16)

    maps = []
    for c in range(NCORES):
        xs = x[c * BL:(c + 1) * BL]                       # [4, T, E]
        xTc = np.ascontiguousarray(
            xs.transpose(2, 0, 1).reshape(E, TOK)).astype(NP16)
        maps.append({"xT": xTc, "w8T": w8T16, "b8": b8, "whh": whh,
                     "selG": selG, "selGT": selGT, "permG": permG})
    return maps


def _run(inputs, trace=False, tmpdir=None):
    nc = _get_nc()
    maps = _prep_core_inputs(**inputs)
    res = run_bass_kernel_spmd(nc, maps, list(range(NCORES)), trace=trace,
                               tmpdir=tmpdir)
    outs = []
    for c in range(NCORES):
        oT = res.results[c]["outT"].astype(np.float32)    # [E, TOK]
        outs.append(oT.reshape(E, BL, T).transpose(1, 2, 0))
    return np.concatenate(outs, axis=0), res


def kernel(**inputs):
    out, _ = _run(inputs, trace=False)
    return out


# revision 17
# speedup vs baseline: 1.1514x; 1.1514x over previous
"""Bidirectional H=1 LSTM attention kernel for Trainium2 (8 NeuronCores).

Model: hs = BiLSTM(x) [B,T,2] -> att = softmax(mean(hs,-1), axis=T) -> out = att[:,:,None]*x
Shapes: B=32, T=4096, E=300, H=1.

v4: bf16 datapath + 2-batch-group software pipeline.
  - Partition layout p = g*64 + d*32 + brel*16 + k (g = batch group of 2,
    d = direction, brel = batch within group, k = chunk).  Each group's scan
    touches one contiguous 64-row range, so group pipelining is legal for
    the compute engines (partition starts 0/32/64/96).
  - Emission interleaves: g0-p1 | g1-p1 + g0-scan | g0-p5 + g1-scan | g1-p5
    at per-block/per-iteration granularity, so the in-order engines overlap
    phases (PE crunches g1 matmuls while DVE/ACT scan g0, output DMA of g0
    runs under g1's scan).
  - fwd/bwd row pairing for the softmax is restored by a 32x32 block-reversal
    permutation matmul on the PE (DMA gathers keep ascending strides).
  - att broadcast to 128 partitions via K=1 PE outer product into PSUM.
"""

import sys

sys.path.insert(0, "/opt/trn_rl_repo")

import numpy as np
from contextlib import ExitStack

import concourse.bass as bass
import concourse.bacc as bacc
import concourse.tile as tile
from concourse import mybir
from concourse.bass_utils import run_bass_kernel_spmd

F32 = mybir.dt.float32
USE_BF16 = True
F16 = mybir.dt.bfloat16 if USE_BF16 else mybir.dt.float16
import ml_dtypes
NP16 = ml_dtypes.bfloat16 if USE_BF16 else np.float16
AF = mybir.ActivationFunctionType
ALU = mybir.AluOpType

NCORES = 8
B, T, E = 32, 4096, 300
BL = B // NCORES          # batches per core
TOK = BL * T              # tokens per core (b-major)
L, W = 256, 32            # chunk len, halo warmup
S = L + W                 # scan steps per chunk
K = T // L                # chunks per (dir, batch)
P = 128                   # partitions = g*64 + d*32 + brel*16 + k
NG = 2                    # batch groups (pipeline stages)
GTOK = TOK // NG          # tokens per group
N_ITER = 4                # fixed-point iterations (validated offline)
PADROW = W + T + W        # padded xg row per batch
CB = 2048                 # token block for loads / p5
NBG = GTOK // CB          # blocks per group (4)
# gate order inside a block row: (i, f, o, g) ; pytorch order is (i, f, g, o)
GATE_PERM = [0, 1, 3, 2]


def _build_nc():
    nc = bacc.Bacc(None, target_bir_lowering=False, debug=False)
    xT = nc.declare_dram_parameter("xT", [E, TOK], F16, isOutput=False)
    w8T = nc.declare_dram_parameter("w8T", [E, 36], F16, isOutput=False)
    b8 = nc.declare_dram_parameter("b8", [36, 1], F32, isOutput=False)
    whh = nc.declare_dram_parameter("whh", [64, 4], F32, isOutput=False)
    selG = nc.declare_dram_parameter("selG", [32, 2], F32, isOutput=False)
    selGT = nc.declare_dram_parameter("selGT", [2, 32], F32, isOutput=False)
    permG = nc.declare_dram_parameter("permG", [32, 32], F16, isOutput=False)
    outT = nc.declare_dram_parameter("outT", [E, TOK], F16, isOutput=True)

    # internal DRAM scratch, split per group to avoid cross-group false
    # dependencies in the tile scheduler: rows (brel)*8 + d*4 + gate
    dxg0 = nc.dram_tensor("dxg0", [16, PADROW], F16)
    dxg1 = nc.dram_tensor("dxg1", [16, PADROW], F16)
    datt_g = [nc.dram_tensor(f"datt{gg}", [1, GTOK], F16) for gg in range(2)]

    with tile.TileContext(nc) as tc, ExitStack() as ctx:
        singles = ctx.enter_context(tc.tile_pool(name="singles", bufs=1))
        stage = ctx.enter_context(tc.tile_pool(name="stage", bufs=2))
        scanp = ctx.enter_context(tc.tile_pool(name="scanp", bufs=1))
        papool = ctx.enter_context(tc.tile_pool(name="papool", bufs=2))
        opool = ctx.enter_context(tc.tile_pool(name="opool", bufs=4))
        o2pool = ctx.enter_context(tc.tile_pool(name="o2pool", bufs=2))
        psA = ctx.enter_context(tc.tile_pool(name="psA", bufs=6, space="PSUM"))
        psB = ctx.enter_context(tc.tile_pool(name="psB", bufs=1, space="PSUM"))

        # ---- constants / resident tiles ----
        w8a = singles.tile([128, 36], F16)
        w8b = singles.tile([128, 36], F16)
        w8c = singles.tile([44, 36], F16)
        nc.gpsimd.dma_start(out=w8a, in_=w8T[0:128, :])
        nc.gpsimd.dma_start(out=w8b, in_=w8T[128:256, :])
        nc.gpsimd.dma_start(out=w8c, in_=w8T[256:300, :])
        b8_sb = singles.tile([36, 1], F32)
        nc.sync.dma_start(out=b8_sb, in_=b8[:, :])
        whh_sb = singles.tile([64, 4], F32)
        nc.sync.dma_start(out=whh_sb, in_=whh[:, :])
        sel_sb = singles.tile([32, 2], F32)
        nc.sync.dma_start(out=sel_sb, in_=selG[:, :])
        selT_sb = singles.tile([2, 32], F32)
        nc.sync.dma_start(out=selT_sb, in_=selGT[:, :])
        perm_sb = singles.tile([32, 32], F16)
        nc.sync.dma_start(out=perm_sb, in_=permG[:, :])

        xT0 = singles.tile([128, TOK], F16)   # e 0..127 resident
        xT1 = singles.tile([128, TOK], F16)   # e 128..255 resident
        xT2 = singles.tile([44, TOK], F16)    # e 256..299 resident

        # zero-pad regions of dxg (halo reads beyond sequence ends)
        zpad = singles.tile([16, W], F16)
        nc.vector.memset(zpad[:, :], 0.0)
        for dx in (dxg0, dxg1):
            nc.sync.dma_start(out=dx[:, 0:W], in_=zpad[:, :])
            nc.sync.dma_start(out=dx[:, W + T:PADROW], in_=zpad[:, :])

        # scan-layout xg per group: rows d*32 + brel*16 + k, cols g*S + s
        xg_g = []
        h_g = []
        for gg in range(2):
            xgt = singles.tile([64, 4 * S], F16, name=f"xg{gg}")
            xg_g.append(xgt)
            hgt = singles.tile([64, S + 1], F16, name=f"h{gg}")
            h_g.append(hgt)
            nc.vector.memset(hgt[:, :], 0.0)

        # ---------------- emission helpers ----------------
        def emit_p1_block(g, i):
            """Load 2048 tokens, matmul xg, evacuate, write dxg, gather."""
            tt = g * NBG + i
            b = (tt * CB) // T
            toff = (tt * CB) % T
            cols = slice(tt * CB, (tt + 1) * CB)
            nc.gpsimd.dma_start(out=xT0[:, cols], in_=xT[0:128, cols])
            nc.sync.dma_start(out=xT1[:, cols], in_=xT[128:256, cols])
            nc.scalar.dma_start(out=xT2[:, cols], in_=xT[256:300, cols])
            st8 = stage.tile([36, CB], F16, tag="st8")
            c0 = tt * CB
            for q in range(4):
                ps = psA.tile([36, 512], F32, tag="ps")
                if q == 0:
                    # touch matmuls: absorb each fresh DMA's semaphore on the
                    # PE clock (Matmult codegen has one sync-wait slot).
                    nc.tensor.matmul(ps[0:2, 0:2], lhsT=xT0[:, c0:c0 + 2],
                                     rhs=xT0[:, c0:c0 + 2], start=True,
                                     stop=True)
                    nc.tensor.matmul(ps[0:2, 2:4], lhsT=xT1[:, c0:c0 + 2],
                                     rhs=xT1[:, c0:c0 + 2], start=True,
                                     stop=True)
                    nc.tensor.matmul(ps[0:2, 4:6], lhsT=xT2[:, c0:c0 + 2],
                                     rhs=xT2[:, c0:c0 + 2], start=True,
                                     stop=True)
                    if tt == 0:
                        nc.tensor.matmul(ps[0:2, 6:8], lhsT=sel_sb[:, 0:2],
                                         rhs=sel_sb[:, 0:2], start=True,
                                         stop=True)
                        nc.tensor.matmul(ps[0:2, 8:10], lhsT=selT_sb[:, 0:2],
                                         rhs=selT_sb[:, 0:2], start=True,
                                         stop=True)
                        nc.tensor.matmul(ps[0:2, 12:14], lhsT=perm_sb[:, 0:2],
                                         rhs=perm_sb[:, 0:2], start=True,
                                         stop=True)
                cs = slice(tt * CB + q * 512, tt * CB + (q + 1) * 512)
                for ci, (wch, xch) in enumerate(
                        [(w8a, xT0), (w8b, xT1), (w8c, xT2)]):
                    nc.tensor.matmul(ps, lhsT=wch, rhs=xch[:, cs],
                                     start=(ci == 0), stop=(ci == 2),
                                     skip_group_check=True)
                # evacuate + bias on ACT (single psA reader keeps matmul WAR
                # to one semaphore)
                nc.scalar.activation(st8[:, q * 512:(q + 1) * 512],
                                     ps, AF.Identity, bias=b8_sb[:, :],
                                     scale=1.0)
            dst0 = W + toff
            dxg = dxg0 if g == 0 else dxg1
            brel = b % 2
            nc.sync.dma_start(out=dxg[brel * 8:brel * 8 + 4, dst0:dst0 + CB],
                              in_=st8[0:4, :])
            # d=1 rows stored time-REVERSED (col W+r holds t=T-1-r)
            strev = stage.tile([4, CB], F16, tag="strev")
            nc.vector.tensor_copy(strev, st8[32:36, ::-1])
            lo = PADROW - CB - dst0
            nc.sync.dma_start(out=dxg[brel * 8 + 4:brel * 8 + 8, lo:lo + CB],
                              in_=strev)
            if toff + CB == T:
                # batch b complete: gather both dirs into scan layout.
                # dst rows d*32 + brel*16 + k; src runs of S elems; bwd rows
                # hold scan chunk K-1-khat (ascending strides; the att stage
                # repairs pairing with a PE permutation).
                base = dxg[:, :]
                for d in range(2):
                    src = bass.AP(
                        tensor=base.tensor,
                        offset=(brel * 8 + d * 4) * PADROW,
                        ap=[[L, K], [PADROW, 4], [1, S]])
                    r0 = d * 32 + brel * 16
                    nc.sync.dma_start(
                        out=xg_g[g][r0:r0 + 16, :].rearrange(
                            "p (g s) -> p g s", g=4),
                        in_=src)

        def emit_scan_iter(g, it):
            xg_t = xg_g[g]
            h_st = h_g[g]
            r = slice(g * 64, (g + 1) * 64)
            if it == 0:
                gsrc = xg_t
            else:
                gsrc = scanp.tile([64, 4 * S], F16, tag=f"gbuf{g}")
                for gg in (0, 3, 1, 2):   # i, g first: unblocks mt earliest
                    nc.vector.scalar_tensor_tensor(
                        out=gsrc[:, gg * S:(gg + 1) * S],
                        in0=h_st[:, 0:S],
                        scalar=whh_sb[:, gg:gg + 1],
                        in1=xg_t[:, gg * S:(gg + 1) * S],
                        op0=ALU.mult, op1=ALU.add)

            def gs(a, b_):
                return gsrc[:, a:b_]
            St = scanp.tile([64, 3 * S], F16, tag=f"St{g}")
            Gt = scanp.tile([64, S], F16, tag=f"Gt{g}")
            mt = scanp.tile([64, S], F16, tag=f"mt{g}")
            ct = scanp.tile([64, S], F16, tag=f"ct{g}")
            tct = scanp.tile([64, S], F16, tag=f"tct{g}")
            nc.scalar.activation(St[:, 0:S], gs(0, S), AF.Sigmoid)
            nc.scalar.activation(Gt, gs(3 * S, 4 * S), AF.Tanh)
            nc.vector.tensor_mul(mt, St[:, 0:S], Gt)
            nc.scalar.activation(St[:, S:2 * S], gs(S, 2 * S), AF.Sigmoid)
            nc.vector.tensor_tensor_scan(
                out=ct, data0=St[:, S:2 * S], data1=mt, initial=0.0,
                op0=ALU.mult, op1=ALU.add)
            nc.scalar.activation(St[:, 2 * S:3 * S], gs(2 * S, 3 * S),
                                 AF.Sigmoid)
            nc.scalar.activation(tct, ct, AF.Tanh)
            nc.vector.tensor_mul(h_st[:, 1:S + 1], St[:, 2 * S:3 * S], tct)

        def emit_att(g):
            h_st = h_g[g]
            h_rev = scanp.tile([32, S + 1], F16, tag=f"hrev{g}")
            nc.vector.tensor_copy(h_rev, h_st[32:64, ::-1])
            # one PSUM bank holds all three small attention results
            psb = psB.tile([32, 512], F32, tag="psb")
            hb_perm = psb[:, 0:L]
            nc.tensor.matmul(hb_perm, lhsT=perm_sb, rhs=h_rev[:, 0:L],
                             start=True, stop=True)
            hsum = scanp.tile([32, L], F16, tag=f"hsum{g}")
            nc.vector.tensor_add(hsum, h_st[0:32, W + 1:S + 1], hb_perm)
            negone = scanp.tile([32, 1], F32, tag=f"negone{g}")
            nc.vector.memset(negone[:, :], -1.0)
            exps = scanp.tile([32, L], F32, tag=f"exps{g}")
            s1 = scanp.tile([32, 1], F32, tag=f"s1{g}")
            # exp(0.5*hsum - 1) in [e^-2, 1]: stable without max-subtraction
            nc.scalar.activation(exps, hsum, AF.Exp, bias=negone[:, :],
                                 scale=0.5, accum_out=s1)
            ps_s = psb[0:2, 300:301]
            nc.tensor.matmul(ps_s, lhsT=sel_sb, rhs=s1, start=True, stop=True)
            r2 = scanp.tile([2, 1], F32, tag=f"r2{g}")
            nc.vector.reciprocal(r2, ps_s)
            ps_r = psb[:, 320:321]
            nc.tensor.matmul(ps_r, lhsT=selT_sb, rhs=r2, start=True, stop=True)
            att_r = scanp.tile([32, L], F16, tag=f"attr{g}")
            nc.vector.tensor_scalar_mul(att_r, exps, ps_r[:, 0:1])
            # flatten rows into token order: row r=(brel*16+k) -> offset r*L
            # (gpsimd queue: keeps the sync queue free for loads/stores)
            nc.gpsimd.dma_start(
                out=datt_g[g][0:1, :].rearrange("p (r s) -> p r s", r=32),
                in_=att_r[:, :])

        def emit_p5_block(g, i):
            tt = g * NBG + i
            cols = slice(tt * CB, (tt + 1) * CB)
            # broadcast att across 128 partitions with a stride-0 SWDGE DMA
            # (gpsimd queue idle in p5; no PE/ACT involvement)
            pa = papool.tile([128, CB], F16, tag="pa")
            nc.gpsimd.dma_start(
                out=pa,
                in_=bass.AP(tensor=datt_g[g][:, :].tensor, offset=i * CB,
                            ap=[[0, 128], [1, CB]]))
            ob0 = opool.tile([128, CB], F16, tag="ob")
            nc.vector.tensor_mul(ob0, xT0[:, cols], pa)
            nc.sync.dma_start(out=outT[0:128, cols], in_=ob0)
            ob1 = opool.tile([128, CB], F16, tag="ob")
            nc.vector.tensor_mul(ob1, xT1[:, cols], pa)
            nc.scalar.dma_start(out=outT[128:256, cols], in_=ob1)
            ob2 = o2pool.tile([44, CB], F16, tag="ob2")
            nc.vector.tensor_mul(ob2, xT2[:, cols], pa[0:44, :])
            nc.scalar.dma_start(out=outT[256:300, cols], in_=ob2)

        # ---------------- pipeline schedule ----------------
        for i in range(NBG):
            emit_p1_block(0, i)
        # 2:1 interleave: scan iterations land early in each engine's
        # in-order stream; att's PE/ACT ops precede g1's last block so the
        # engines reach them as soon as dependencies allow.
        emit_p1_block(1, 0)
        emit_scan_iter(0, 0)
        emit_scan_iter(0, 1)
        emit_p1_block(1, 1)
        emit_scan_iter(0, 2)
        emit_scan_iter(0, 3)
        emit_p1_block(1, 2)
        emit_att(0)
        emit_p1_block(1, 3)
        emit_p5_block(0, 0)
        emit_scan_iter(1, 0)
        emit_scan_iter(1, 1)
        emit_p5_block(0, 1)
        emit_scan_iter(1, 2)
        emit_scan_iter(1, 3)
        emit_p5_block(0, 2)
        emit_att(1)
        emit_p5_block(0, 3)
        for i in range(NBG):
            emit_p5_block(1, i)

    return nc


_NC = None


def _get_nc():
    global _NC
    if _NC is None:
        _NC = _build_nc()
        _NC.finalize()
    return _NC


def _prep_core_inputs(x, w_ih_f, w_hh_f, b_ih_f, b_hh_f,
                      w_ih_b, w_hh_b, b_ih_b, b_hh_b):
    """Build the per-core input maps."""
    w8T = np.zeros((E, 36), np.float32)
    b8 = np.zeros((36, 1), np.float32)
    whh = np.zeros((64, 4), np.float32)
    for d, (wi, wh, bi, bh) in enumerate(
            [(w_ih_f, w_hh_f, b_ih_f, b_hh_f),
             (w_ih_b, w_hh_b, b_ih_b, b_hh_b)]):
        for j, gp in enumerate(GATE_PERM):
            w8T[:, d * 32 + j] = wi[gp, :]
            b8[d * 32 + j, 0] = bi[gp] + bh[gp]
            # rows p = d*32 + brel*16 + k (per group): d = p // 32
            whh[d * 32:(d + 1) * 32, j] = wh[gp, 0]
    selG = np.zeros((32, 2), np.float32)
    for r in range(32):
        selG[r, r // 16] = 1.0
    selGT = np.ascontiguousarray(selG.T)
    permG = np.zeros((32, 32), NP16)
    for bb in range(2):
        for i in range(16):
            permG[bb * 16 + i, bb * 16 + 15 - i] = 1.0
    w8T16 = w8T.astype(NP16)

    maps = []
    for c in range(NCORES):
        xs = x[c * BL:(c + 1) * BL]                       # [4, T, E]
        xTc = np.ascontiguousarray(
            xs.transpose(2, 0, 1).reshape(E, TOK)).astype(NP16)
        maps.append({"xT": xTc, "w8T": w8T16, "b8": b8, "whh": whh,
                     "selG": selG, "selGT": selGT, "permG": permG})
    return maps


def _run(inputs, trace=False, tmpdir=None):
    nc = _get_nc()
    maps = _prep_core_inputs(**inputs)
    res = run_bass_kernel_spmd(nc, maps, list(range(NCORES)), trace=trace,
                               tmpdir=tmpdir)
    outs = []
    for c in range(NCORES):
        oT = res.results[c]["outT"].astype(np.float32)    # [E, TOK]
        outs.append(oT.reshape(E, BL, T).transpose(1, 2, 0))
    return np.concatenate(outs, axis=0), res


def kernel(**inputs):
    out, _ = _run(inputs, trace=False)
    return out


# revision 19
# speedup vs baseline: 1.2428x; 1.0794x over previous
"""Bidirectional H=1 LSTM attention kernel for Trainium2 (8 NeuronCores).

Model: hs = BiLSTM(x) [B,T,2] -> att = softmax(mean(hs,-1), axis=T) -> out = att[:,:,None]*x
Shapes: B=32, T=4096, E=300, H=1.

v4: bf16 datapath + 2-batch-group software pipeline.
  - Partition layout p = g*64 + d*32 + brel*16 + k (g = batch group of 2,
    d = direction, brel = batch within group, k = chunk).  Each group's scan
    touches one contiguous 64-row range, so group pipelining is legal for
    the compute engines (partition starts 0/32/64/96).
  - Emission interleaves: g0-p1 | g1-p1 + g0-scan | g0-p5 + g1-scan | g1-p5
    at per-block/per-iteration granularity, so the in-order engines overlap
    phases (PE crunches g1 matmuls while DVE/ACT scan g0, output DMA of g0
    runs under g1's scan).
  - fwd/bwd row pairing for the softmax is restored by a 32x32 block-reversal
    permutation matmul on the PE (DMA gathers keep ascending strides).
  - att broadcast to 128 partitions via K=1 PE outer product into PSUM.
"""

import sys

sys.path.insert(0, "/opt/trn_rl_repo")

import numpy as np
from contextlib import ExitStack

import concourse.bass as bass
import concourse.bacc as bacc
import concourse.tile as tile
from concourse import mybir
from concourse.bass_utils import run_bass_kernel_spmd

F32 = mybir.dt.float32
USE_BF16 = True
F16 = mybir.dt.bfloat16 if USE_BF16 else mybir.dt.float16
import ml_dtypes
NP16 = ml_dtypes.bfloat16 if USE_BF16 else np.float16
AF = mybir.ActivationFunctionType
ALU = mybir.AluOpType

NCORES = 8
B, T, E = 32, 4096, 300
BL = B // NCORES          # batches per core
TOK = BL * T              # tokens per core (b-major)
L, W = 256, 32            # chunk len, halo warmup
S = L + W                 # scan steps per chunk
K = T // L                # chunks per (dir, batch)
P = 128                   # partitions = g*64 + d*32 + brel*16 + k
NG = 2                    # batch groups (pipeline stages)
GTOK = TOK // NG          # tokens per group
N_ITER = 4                # fixed-point iterations (validated offline)
PADROW = W + T + W        # padded xg row per batch
CB = 4096                 # token block for loads / p5 (one batch per block)
NBG = GTOK // CB          # blocks per group (4)
# gate order inside a block row: (i, f, o, g) ; pytorch order is (i, f, g, o)
GATE_PERM = [0, 1, 3, 2]


def _build_nc():
    nc = bacc.Bacc(None, target_bir_lowering=False, debug=False)
    xT = nc.declare_dram_parameter("xT", [E, TOK], F16, isOutput=False)
    w8T = nc.declare_dram_parameter("w8T", [E, 36], F16, isOutput=False)
    b8 = nc.declare_dram_parameter("b8", [36, 1], F32, isOutput=False)
    whh = nc.declare_dram_parameter("whh", [64, 4], F32, isOutput=False)
    selG = nc.declare_dram_parameter("selG", [32, 2], F32, isOutput=False)
    selGT = nc.declare_dram_parameter("selGT", [2, 32], F32, isOutput=False)
    permG = nc.declare_dram_parameter("permG", [32, 32], F16, isOutput=False)
    outT = nc.declare_dram_parameter("outT", [E, TOK], F16, isOutput=True)

    # internal DRAM scratch, split per group to avoid cross-group false
    # dependencies in the tile scheduler: rows (brel)*8 + d*4 + gate
    dxg0 = nc.dram_tensor("dxg0", [16, PADROW], F16)
    dxg1 = nc.dram_tensor("dxg1", [16, PADROW], F16)
    datt_g = [nc.dram_tensor(f"datt{gg}", [1, GTOK], F16) for gg in range(2)]

    with tile.TileContext(nc) as tc, ExitStack() as ctx:
        singles = ctx.enter_context(tc.tile_pool(name="singles", bufs=1))
        stage = ctx.enter_context(tc.tile_pool(name="stage", bufs=2))
        scanp = ctx.enter_context(tc.tile_pool(name="scanp", bufs=1))
        papool = ctx.enter_context(tc.tile_pool(name="papool", bufs=2))
        opool = ctx.enter_context(tc.tile_pool(name="opool", bufs=3))
        o2pool = ctx.enter_context(tc.tile_pool(name="o2pool", bufs=1))
        psA = ctx.enter_context(tc.tile_pool(name="psA", bufs=7, space="PSUM"))
        psB = ctx.enter_context(tc.tile_pool(name="psB", bufs=1, space="PSUM"))

        # ---- constants / resident tiles ----
        w8a = singles.tile([128, 36], F16)
        w8b = singles.tile([128, 36], F16)
        w8c = singles.tile([44, 36], F16)
        nc.gpsimd.dma_start(out=w8a, in_=w8T[0:128, :])
        nc.gpsimd.dma_start(out=w8b, in_=w8T[128:256, :])
        nc.gpsimd.dma_start(out=w8c, in_=w8T[256:300, :])
        b8_sb = singles.tile([36, 1], F32)
        nc.sync.dma_start(out=b8_sb, in_=b8[:, :])
        whh_sb = singles.tile([64, 4], F32)
        nc.sync.dma_start(out=whh_sb, in_=whh[:, :])
        sel_sb = singles.tile([32, 2], F32)
        nc.sync.dma_start(out=sel_sb, in_=selG[:, :])
        selT_sb = singles.tile([2, 32], F32)
        nc.sync.dma_start(out=selT_sb, in_=selGT[:, :])
        perm_sb = singles.tile([32, 32], F16)
        nc.sync.dma_start(out=perm_sb, in_=permG[:, :])

        xT0 = singles.tile([128, TOK], F16)   # e 0..127 resident
        xT1 = singles.tile([128, TOK], F16)   # e 128..255 resident
        xT2 = singles.tile([44, TOK], F16)    # e 256..299 resident

        # zero-pad regions of dxg (halo reads beyond sequence ends)
        zpad = singles.tile([16, W], F16)
        nc.vector.memset(zpad[:, :], 0.0)
        for dx in (dxg0, dxg1):
            nc.sync.dma_start(out=dx[:, 0:W], in_=zpad[:, :])
            nc.sync.dma_start(out=dx[:, W + T:PADROW], in_=zpad[:, :])

        # scan-layout xg per group: rows d*32 + brel*16 + k, cols g*S + s
        xg_g = []
        h_g = []
        for gg in range(2):
            xgt = singles.tile([64, 4 * S], F16, name=f"xg{gg}")
            xg_g.append(xgt)
            hgt = singles.tile([64, S + 1], F16, name=f"h{gg}")
            h_g.append(hgt)
            nc.vector.memset(hgt[:, :], 0.0)

        # ---------------- emission helpers ----------------
        def emit_p1_block(g, i):
            """Load 2048 tokens, matmul xg, evacuate, write dxg, gather."""
            tt = g * NBG + i
            b = (tt * CB) // T
            toff = (tt * CB) % T
            cols = slice(tt * CB, (tt + 1) * CB)
            nc.gpsimd.dma_start(out=xT0[:, cols], in_=xT[0:128, cols])
            nc.sync.dma_start(out=xT1[:, cols], in_=xT[128:256, cols])
            nc.gpsimd.dma_start(out=xT2[:, cols], in_=xT[256:300, cols])
            st8 = stage.tile([36, CB], F16, tag="st8")
            c0 = tt * CB
            for q in range(8):
                ps = psA.tile([36, 512], F32, tag="ps")
                if q == 0:
                    # touch matmuls: absorb each fresh DMA's semaphore on the
                    # PE clock (Matmult codegen has one sync-wait slot).
                    nc.tensor.matmul(ps[0:2, 0:2], lhsT=xT0[:, c0:c0 + 2],
                                     rhs=xT0[:, c0:c0 + 2], start=True,
                                     stop=True)
                    nc.tensor.matmul(ps[0:2, 2:4], lhsT=xT1[:, c0:c0 + 2],
                                     rhs=xT1[:, c0:c0 + 2], start=True,
                                     stop=True)
                    nc.tensor.matmul(ps[0:2, 4:6], lhsT=xT2[:, c0:c0 + 2],
                                     rhs=xT2[:, c0:c0 + 2], start=True,
                                     stop=True)
                    if tt == 0:
                        nc.tensor.matmul(ps[0:2, 6:8], lhsT=sel_sb[:, 0:2],
                                         rhs=sel_sb[:, 0:2], start=True,
                                         stop=True)
                        nc.tensor.matmul(ps[0:2, 8:10], lhsT=selT_sb[:, 0:2],
                                         rhs=selT_sb[:, 0:2], start=True,
                                         stop=True)
                        nc.tensor.matmul(ps[0:2, 12:14], lhsT=perm_sb[:, 0:2],
                                         rhs=perm_sb[:, 0:2], start=True,
                                         stop=True)
                cs = slice(tt * CB + q * 512, tt * CB + (q + 1) * 512)
                for ci, (wch, xch) in enumerate(
                        [(w8a, xT0), (w8b, xT1), (w8c, xT2)]):
                    nc.tensor.matmul(ps, lhsT=wch, rhs=xch[:, cs],
                                     start=(ci == 0), stop=(ci == 2),
                                     skip_group_check=True)
                # evacuate + bias on ACT (single psA reader keeps matmul WAR
                # to one semaphore)
                nc.scalar.activation(st8[:, q * 512:(q + 1) * 512],
                                     ps, AF.Identity, bias=b8_sb[:, :],
                                     scale=1.0)
            dst0 = W + toff
            dxg = dxg0 if g == 0 else dxg1
            brel = b % 2
            nc.sync.dma_start(out=dxg[brel * 8:brel * 8 + 4, dst0:dst0 + CB],
                              in_=st8[0:4, :])
            # d=1 rows stored time-REVERSED (col W+r holds t=T-1-r)
            strev = stage.tile([4, CB], F16, tag="strev")
            nc.vector.tensor_copy(strev, st8[32:36, ::-1])
            lo = PADROW - CB - dst0
            nc.sync.dma_start(out=dxg[brel * 8 + 4:brel * 8 + 8, lo:lo + CB],
                              in_=strev)
            if toff + CB == T:
                # batch b complete: gather both dirs into scan layout.
                # dst rows d*32 + brel*16 + k; src runs of S elems; bwd rows
                # hold scan chunk K-1-khat (ascending strides; the att stage
                # repairs pairing with a PE permutation).
                base = dxg[:, :]
                for d in range(2):
                    src = bass.AP(
                        tensor=base.tensor,
                        offset=(brel * 8 + d * 4) * PADROW,
                        ap=[[L, K], [PADROW, 4], [1, S]])
                    r0 = d * 32 + brel * 16
                    nc.sync.dma_start(
                        out=xg_g[g][r0:r0 + 16, :].rearrange(
                            "p (g s) -> p g s", g=4),
                        in_=src)

        def emit_scan_iter(g, it):
            xg_t = xg_g[g]
            h_st = h_g[g]
            r = slice(g * 64, (g + 1) * 64)
            if it == 0:
                gsrc = xg_t
            else:
                gsrc = scanp.tile([64, 4 * S], F16, tag=f"gbuf{g}")
                for gg in (0, 3, 1, 2):   # i, g first: unblocks mt earliest
                    nc.vector.scalar_tensor_tensor(
                        out=gsrc[:, gg * S:(gg + 1) * S],
                        in0=h_st[:, 0:S],
                        scalar=whh_sb[:, gg:gg + 1],
                        in1=xg_t[:, gg * S:(gg + 1) * S],
                        op0=ALU.mult, op1=ALU.add)

            def gs(a, b_):
                return gsrc[:, a:b_]
            St = scanp.tile([64, 3 * S], F16, tag=f"St{g}")
            Gt = scanp.tile([64, S], F16, tag=f"Gt{g}")
            mt = scanp.tile([64, S], F16, tag=f"mt{g}")
            ct = scanp.tile([64, S], F16, tag=f"ct{g}")
            tct = scanp.tile([64, S], F16, tag=f"tct{g}")
            nc.scalar.activation(St[:, 0:S], gs(0, S), AF.Sigmoid)
            nc.scalar.activation(Gt, gs(3 * S, 4 * S), AF.Tanh)
            nc.vector.tensor_mul(mt, St[:, 0:S], Gt)
            nc.scalar.activation(St[:, S:2 * S], gs(S, 2 * S), AF.Sigmoid)
            nc.vector.tensor_tensor_scan(
                out=ct, data0=St[:, S:2 * S], data1=mt, initial=0.0,
                op0=ALU.mult, op1=ALU.add)
            nc.scalar.activation(St[:, 2 * S:3 * S], gs(2 * S, 3 * S),
                                 AF.Sigmoid)
            nc.scalar.activation(tct, ct, AF.Tanh)
            nc.vector.tensor_mul(h_st[:, 1:S + 1], St[:, 2 * S:3 * S], tct)

        def emit_att(g):
            h_st = h_g[g]
            h_rev = scanp.tile([32, S + 1], F16, tag=f"hrev{g}")
            nc.vector.tensor_copy(h_rev, h_st[32:64, ::-1])
            # one PSUM bank holds all three small attention results
            psb = psB.tile([32, 512], F32, tag="psb")
            hb_perm = psb[:, 0:L]
            nc.tensor.matmul(hb_perm, lhsT=perm_sb, rhs=h_rev[:, 0:L],
                             start=True, stop=True)
            hsum = scanp.tile([32, L], F16, tag=f"hsum{g}")
            nc.vector.tensor_add(hsum, h_st[0:32, W + 1:S + 1], hb_perm)
            negone = scanp.tile([32, 1], F32, tag=f"negone{g}")
            nc.vector.memset(negone[:, :], -1.0)
            exps = scanp.tile([32, L], F32, tag=f"exps{g}")
            s1 = scanp.tile([32, 1], F32, tag=f"s1{g}")
            # exp(0.5*hsum - 1) in [e^-2, 1]: stable without max-subtraction
            nc.scalar.activation(exps, hsum, AF.Exp, bias=negone[:, :],
                                 scale=0.5, accum_out=s1)
            ps_s = psb[0:2, 300:301]
            nc.tensor.matmul(ps_s, lhsT=sel_sb, rhs=s1, start=True, stop=True)
            r2 = scanp.tile([2, 1], F32, tag=f"r2{g}")
            nc.vector.reciprocal(r2, ps_s)
            ps_r = psb[:, 320:321]
            nc.tensor.matmul(ps_r, lhsT=selT_sb, rhs=r2, start=True, stop=True)
            att_r = scanp.tile([32, L], F16, tag=f"attr{g}")
            nc.vector.tensor_scalar_mul(att_r, exps, ps_r[:, 0:1])
            # flatten rows into token order: row r=(brel*16+k) -> offset r*L
            # (gpsimd queue: keeps the sync queue free for loads/stores)
            nc.gpsimd.dma_start(
                out=datt_g[g][0:1, :].rearrange("p (r s) -> p r s", r=32),
                in_=att_r[:, :])

        def emit_p5_block(g, i):
            tt = g * NBG + i
            cols = slice(tt * CB, (tt + 1) * CB)
            # broadcast att across 128 partitions with a stride-0 SWDGE DMA
            # (gpsimd queue idle in p5; no PE/ACT involvement)
            pa = papool.tile([128, CB], F16, tag="pa")
            nc.gpsimd.dma_start(
                out=pa,
                in_=bass.AP(tensor=datt_g[g][:, :].tensor, offset=i * CB,
                            ap=[[0, 128], [1, CB]]))
            ob0 = opool.tile([128, CB], F16, tag="ob")
            nc.vector.tensor_mul(ob0, xT0[:, cols], pa)
            nc.sync.dma_start(out=outT[0:128, cols], in_=ob0)
            ob1 = opool.tile([128, CB], F16, tag="ob")
            nc.vector.tensor_mul(ob1, xT1[:, cols], pa)
            nc.scalar.dma_start(out=outT[128:256, cols], in_=ob1)
            ob2 = o2pool.tile([44, CB], F16, tag="ob2")
            nc.vector.tensor_mul(ob2, xT2[:, cols], pa[0:44, :])
            nc.scalar.dma_start(out=outT[256:300, cols], in_=ob2)

        # ---------------- pipeline schedule ----------------
        for i in range(NBG):
            emit_p1_block(0, i)
        # 2:1 interleave: scan iterations land early in each engine's
        # in-order stream; att's PE/ACT ops precede later blocks so the
        # engines reach them as soon as dependencies allow.
        emit_p1_block(1, 0)
        emit_scan_iter(0, 0)
        emit_scan_iter(0, 1)
        emit_p1_block(1, 1)
        emit_scan_iter(0, 2)
        emit_scan_iter(0, 3)
        emit_att(0)
        emit_p5_block(0, 0)
        emit_scan_iter(1, 0)
        emit_scan_iter(1, 1)
        emit_p5_block(0, 1)
        emit_scan_iter(1, 2)
        emit_scan_iter(1, 3)
        emit_att(1)
        emit_p5_block(1, 0)
        emit_p5_block(1, 1)

    return nc


_NC = None


def _get_nc():
    global _NC
    if _NC is None:
        _NC = _build_nc()
        _NC.finalize()
    return _NC


def _prep_core_inputs(x, w_ih_f, w_hh_f, b_ih_f, b_hh_f,
                      w_ih_b, w_hh_b, b_ih_b, b_hh_b):
    """Build the per-core input maps."""
    w8T = np.zeros((E, 36), np.float32)
    b8 = np.zeros((36, 1), np.float32)
    whh = np.zeros((64, 4), np.float32)
    for d, (wi, wh, bi, bh) in enumerate(
            [(w_ih_f, w_hh_f, b_ih_f, b_hh_f),
             (w_ih_b, w_hh_b, b_ih_b, b_hh_b)]):
        for j, gp in enumerate(GATE_PERM):
            w8T[:, d * 32 + j] = wi[gp, :]
            b8[d * 32 + j, 0] = bi[gp] + bh[gp]
            # rows p = d*32 + brel*16 + k (per group): d = p // 32
            whh[d * 32:(d + 1) * 32, j] = wh[gp, 0]
    selG = np.zeros((32, 2), np.float32)
    for r in range(32):
        selG[r, r // 16] = 1.0
    selGT = np.ascontiguousarray(selG.T)
    permG = np.zeros((32, 32), NP16)
    for bb in range(2):
        for i in range(16):
            permG[bb * 16 + i, bb * 16 + 15 - i] = 1.0
    w8T16 = w8T.astype(NP16)

    maps = []
    for c in range(NCORES):
        xs = x[c * BL:(c + 1) * BL]                       # [4, T, E]
        xTc = np.ascontiguousarray(
            xs.transpose(2, 0, 1).reshape(E, TOK)).astype(NP16)
        maps.append({"xT": xTc, "w8T": w8T16, "b8": b8, "whh": whh,
                     "selG": selG, "selGT": selGT, "permG": permG})
    return maps


def _run(inputs, trace=False, tmpdir=None):
    nc = _get_nc()
    maps = _prep_core_inputs(**inputs)
    res = run_bass_kernel_spmd(nc, maps, list(range(NCORES)), trace=trace,
                               tmpdir=tmpdir)
    outs = []
    for c in range(NCORES):
        oT = res.results[c]["outT"].astype(np.float32)    # [E, TOK]
        outs.append(oT.reshape(E, BL, T).transpose(1, 2, 0))
    return np.concatenate(outs, axis=0), res


def kernel(**inputs):
    out, _ = _run(inputs, trace=False)
    return out


# revision 20
# speedup vs baseline: 1.2577x; 1.0120x over previous
"""Bidirectional H=1 LSTM attention kernel for Trainium2 (8 NeuronCores).

Model: hs = BiLSTM(x) [B,T,2] -> att = softmax(mean(hs,-1), axis=T) -> out = att[:,:,None]*x
Shapes: B=32, T=4096, E=300, H=1.

v4: bf16 datapath + 2-batch-group software pipeline.
  - Partition layout p = g*64 + d*32 + brel*16 + k (g = batch group of 2,
    d = direction, brel = batch within group, k = chunk).  Each group's scan
    touches one contiguous 64-row range, so group pipelining is legal for
    the compute engines (partition starts 0/32/64/96).
  - Emission interleaves: g0-p1 | g1-p1 + g0-scan | g0-p5 + g1-scan | g1-p5
    at per-block/per-iteration granularity, so the in-order engines overlap
    phases (PE crunches g1 matmuls while DVE/ACT scan g0, output DMA of g0
    runs under g1's scan).
  - fwd/bwd row pairing for the softmax is restored by a 32x32 block-reversal
    permutation matmul on the PE (DMA gathers keep ascending strides).
  - att broadcast to 128 partitions via K=1 PE outer product into PSUM.
"""

import sys

sys.path.insert(0, "/opt/trn_rl_repo")

import numpy as np
from contextlib import ExitStack

import concourse.bass as bass
import concourse.bacc as bacc
import concourse.tile as tile
from concourse import mybir
from concourse.bass_utils import run_bass_kernel_spmd

F32 = mybir.dt.float32
USE_BF16 = True
F16 = mybir.dt.bfloat16 if USE_BF16 else mybir.dt.float16
import ml_dtypes
NP16 = ml_dtypes.bfloat16 if USE_BF16 else np.float16
AF = mybir.ActivationFunctionType
ALU = mybir.AluOpType

NCORES = 8
B, T, E = 32, 4096, 300
BL = B // NCORES          # batches per core
TOK = BL * T              # tokens per core (b-major)
L, W = 256, 32            # chunk len, halo warmup
S = L + W                 # scan steps per chunk
K = T // L                # chunks per (dir, batch)
P = 128                   # partitions = g*64 + d*32 + brel*16 + k
NG = 2                    # batch groups (pipeline stages)
GTOK = TOK // NG          # tokens per group
N_ITER = 4                # fixed-point iterations (validated offline)
PADROW = W + T + W        # padded xg row per batch
CB = 4096                 # token block for loads (one batch per block)
P5CB = 2048               # token block for phase 5
NP5 = GTOK // P5CB        # p5 blocks per group (4)
NBG = GTOK // CB          # blocks per group (4)
# gate order inside a block row: (i, f, o, g) ; pytorch order is (i, f, g, o)
GATE_PERM = [0, 1, 3, 2]


def _build_nc():
    nc = bacc.Bacc(None, target_bir_lowering=False, debug=False)
    xT = nc.declare_dram_parameter("xT", [E, TOK], F16, isOutput=False)
    w8T = nc.declare_dram_parameter("w8T", [E, 36], F16, isOutput=False)
    b8 = nc.declare_dram_parameter("b8", [36, 1], F32, isOutput=False)
    whh = nc.declare_dram_parameter("whh", [64, 4], F32, isOutput=False)
    selG = nc.declare_dram_parameter("selG", [32, 2], F32, isOutput=False)
    selGT = nc.declare_dram_parameter("selGT", [2, 32], F32, isOutput=False)
    permG = nc.declare_dram_parameter("permG", [32, 32], F16, isOutput=False)
    outT = nc.declare_dram_parameter("outT", [E, TOK], F16, isOutput=True)

    # internal DRAM scratch, split per group to avoid cross-group false
    # dependencies in the tile scheduler: rows (brel)*8 + d*4 + gate
    dxg0 = nc.dram_tensor("dxg0", [16, PADROW], F16)
    dxg1 = nc.dram_tensor("dxg1", [16, PADROW], F16)
    datt_g = [nc.dram_tensor(f"datt{gg}", [1, GTOK], F16) for gg in range(2)]

    with tile.TileContext(nc) as tc, ExitStack() as ctx:
        singles = ctx.enter_context(tc.tile_pool(name="singles", bufs=1))
        stage = ctx.enter_context(tc.tile_pool(name="stage", bufs=2))
        scanp = ctx.enter_context(tc.tile_pool(name="scanp", bufs=1))
        papool = ctx.enter_context(tc.tile_pool(name="papool", bufs=4))
        opool = ctx.enter_context(tc.tile_pool(name="opool", bufs=6))
        o2pool = ctx.enter_context(tc.tile_pool(name="o2pool", bufs=2))
        psA = ctx.enter_context(tc.tile_pool(name="psA", bufs=7, space="PSUM"))
        psB = ctx.enter_context(tc.tile_pool(name="psB", bufs=1, space="PSUM"))

        # ---- constants / resident tiles ----
        w8a = singles.tile([128, 36], F16)
        w8b = singles.tile([128, 36], F16)
        w8c = singles.tile([44, 36], F16)
        nc.gpsimd.dma_start(out=w8a, in_=w8T[0:128, :])
        nc.gpsimd.dma_start(out=w8b, in_=w8T[128:256, :])
        nc.gpsimd.dma_start(out=w8c, in_=w8T[256:300, :])
        b8_sb = singles.tile([36, 1], F32)
        nc.sync.dma_start(out=b8_sb, in_=b8[:, :])
        whh_sb = singles.tile([64, 4], F32)
        nc.sync.dma_start(out=whh_sb, in_=whh[:, :])
        sel_sb = singles.tile([32, 2], F32)
        nc.sync.dma_start(out=sel_sb, in_=selG[:, :])
        selT_sb = singles.tile([2, 32], F32)
        nc.sync.dma_start(out=selT_sb, in_=selGT[:, :])
        perm_sb = singles.tile([32, 32], F16)
        nc.sync.dma_start(out=perm_sb, in_=permG[:, :])

        xT0 = singles.tile([128, TOK], F16)   # e 0..127 resident
        xT1 = singles.tile([128, TOK], F16)   # e 128..255 resident
        xT2 = singles.tile([44, TOK], F16)    # e 256..299 resident

        # zero-pad regions of dxg (halo reads beyond sequence ends)
        zpad = singles.tile([16, W], F16)
        nc.vector.memset(zpad[:, :], 0.0)
        for dx in (dxg0, dxg1):
            nc.sync.dma_start(out=dx[:, 0:W], in_=zpad[:, :])
            nc.sync.dma_start(out=dx[:, W + T:PADROW], in_=zpad[:, :])

        # PE warm-up: junk matmuls during the first loads so the tensor
        # engine reaches full p-state before the real xg matmuls arrive.
        junk = singles.tile([128, 512], F16)
        nc.vector.memset(junk[:, :], 0.0)
        for w in range(12):
            psw = psA.tile([36, 512], F32, tag="ps")
            nc.tensor.matmul(psw, lhsT=junk[:, 0:36], rhs=junk,
                             start=True, stop=True)

        # scan-layout xg per group: rows d*32 + brel*16 + k, cols g*S + s
        xg_g = []
        h_g = []
        for gg in range(2):
            xgt = singles.tile([64, 4 * S], F16, name=f"xg{gg}")
            xg_g.append(xgt)
            hgt = singles.tile([64, S + 1], F16, name=f"h{gg}")
            h_g.append(hgt)
            nc.vector.memset(hgt[:, :], 0.0)

        # ---------------- emission helpers ----------------
        def emit_p1_block(g, i):
            """Load 2048 tokens, matmul xg, evacuate, write dxg, gather."""
            tt = g * NBG + i
            b = (tt * CB) // T
            toff = (tt * CB) % T
            cols = slice(tt * CB, (tt + 1) * CB)
            nc.gpsimd.dma_start(out=xT0[:, cols], in_=xT[0:128, cols])
            nc.sync.dma_start(out=xT1[:, cols], in_=xT[128:256, cols])
            nc.gpsimd.dma_start(out=xT2[:, cols], in_=xT[256:300, cols])
            st8 = stage.tile([36, CB], F16, tag="st8")
            c0 = tt * CB
            for q in range(8):
                ps = psA.tile([36, 512], F32, tag="ps")
                if q == 0:
                    # touch matmuls: absorb each fresh DMA's semaphore on the
                    # PE clock (Matmult codegen has one sync-wait slot).
                    nc.tensor.matmul(ps[0:2, 0:2], lhsT=xT0[:, c0:c0 + 2],
                                     rhs=xT0[:, c0:c0 + 2], start=True,
                                     stop=True)
                    nc.tensor.matmul(ps[0:2, 2:4], lhsT=xT1[:, c0:c0 + 2],
                                     rhs=xT1[:, c0:c0 + 2], start=True,
                                     stop=True)
                    nc.tensor.matmul(ps[0:2, 4:6], lhsT=xT2[:, c0:c0 + 2],
                                     rhs=xT2[:, c0:c0 + 2], start=True,
                                     stop=True)
                    if tt == 0:
                        nc.tensor.matmul(ps[0:2, 6:8], lhsT=sel_sb[:, 0:2],
                                         rhs=sel_sb[:, 0:2], start=True,
                                         stop=True)
                        nc.tensor.matmul(ps[0:2, 8:10], lhsT=selT_sb[:, 0:2],
                                         rhs=selT_sb[:, 0:2], start=True,
                                         stop=True)
                        nc.tensor.matmul(ps[0:2, 12:14], lhsT=perm_sb[:, 0:2],
                                         rhs=perm_sb[:, 0:2], start=True,
                                         stop=True)
                cs = slice(tt * CB + q * 512, tt * CB + (q + 1) * 512)
                for ci, (wch, xch) in enumerate(
                        [(w8a, xT0), (w8b, xT1), (w8c, xT2)]):
                    nc.tensor.matmul(ps, lhsT=wch, rhs=xch[:, cs],
                                     start=(ci == 0), stop=(ci == 2),
                                     skip_group_check=True)
                # evacuate + bias on ACT (single psA reader keeps matmul WAR
                # to one semaphore)
                nc.scalar.activation(st8[:, q * 512:(q + 1) * 512],
                                     ps, AF.Identity, bias=b8_sb[:, :],
                                     scale=1.0)
            dst0 = W + toff
            dxg = dxg0 if g == 0 else dxg1
            brel = b % 2
            nc.sync.dma_start(out=dxg[brel * 8:brel * 8 + 4, dst0:dst0 + CB],
                              in_=st8[0:4, :])
            # d=1 rows stored time-REVERSED (col W+r holds t=T-1-r)
            strev = stage.tile([4, CB], F16, tag="strev")
            nc.vector.tensor_copy(strev, st8[32:36, ::-1])
            lo = PADROW - CB - dst0
            nc.sync.dma_start(out=dxg[brel * 8 + 4:brel * 8 + 8, lo:lo + CB],
                              in_=strev)
            if toff + CB == T:
                # batch b complete: gather both dirs into scan layout.
                # dst rows d*32 + brel*16 + k; src runs of S elems; bwd rows
                # hold scan chunk K-1-khat (ascending strides; the att stage
                # repairs pairing with a PE permutation).
                base = dxg[:, :]
                for d in range(2):
                    src = bass.AP(
                        tensor=base.tensor,
                        offset=(brel * 8 + d * 4) * PADROW,
                        ap=[[L, K], [PADROW, 4], [1, S]])
                    r0 = d * 32 + brel * 16
                    nc.sync.dma_start(
                        out=xg_g[g][r0:r0 + 16, :].rearrange(
                            "p (g s) -> p g s", g=4),
                        in_=src)

        def emit_scan_iter(g, it):
            xg_t = xg_g[g]
            h_st = h_g[g]
            r = slice(g * 64, (g + 1) * 64)
            if it == 0:
                gsrc = xg_t
            else:
                gsrc = scanp.tile([64, 4 * S], F16, tag=f"gbuf{g}")
                for gg in (0, 3, 1, 2):   # i, g first: unblocks mt earliest
                    nc.vector.scalar_tensor_tensor(
                        out=gsrc[:, gg * S:(gg + 1) * S],
                        in0=h_st[:, 0:S],
                        scalar=whh_sb[:, gg:gg + 1],
                        in1=xg_t[:, gg * S:(gg + 1) * S],
                        op0=ALU.mult, op1=ALU.add)

            def gs(a, b_):
                return gsrc[:, a:b_]
            St = scanp.tile([64, 3 * S], F16, tag=f"St{g}")
            Gt = scanp.tile([64, S], F16, tag=f"Gt{g}")
            mt = scanp.tile([64, S], F16, tag=f"mt{g}")
            ct = scanp.tile([64, S], F16, tag=f"ct{g}")
            tct = scanp.tile([64, S], F16, tag=f"tct{g}")
            nc.scalar.activation(St[:, 0:S], gs(0, S), AF.Sigmoid)
            nc.scalar.activation(Gt, gs(3 * S, 4 * S), AF.Tanh)
            nc.vector.tensor_mul(mt, St[:, 0:S], Gt)
            nc.scalar.activation(St[:, S:2 * S], gs(S, 2 * S), AF.Sigmoid)
            nc.vector.tensor_tensor_scan(
                out=ct, data0=St[:, S:2 * S], data1=mt, initial=0.0,
                op0=ALU.mult, op1=ALU.add)
            nc.scalar.activation(St[:, 2 * S:3 * S], gs(2 * S, 3 * S),
                                 AF.Sigmoid)
            nc.scalar.activation(tct, ct, AF.Tanh)
            nc.vector.tensor_mul(h_st[:, 1:S + 1], St[:, 2 * S:3 * S], tct)

        def emit_att(g):
            h_st = h_g[g]
            h_rev = scanp.tile([32, S + 1], F16, tag=f"hrev{g}")
            nc.vector.tensor_copy(h_rev, h_st[32:64, ::-1])
            # one PSUM bank holds all three small attention results
            psb = psB.tile([32, 512], F32, tag="psb")
            hb_perm = psb[:, 0:L]
            nc.tensor.matmul(hb_perm, lhsT=perm_sb, rhs=h_rev[:, 0:L],
                             start=True, stop=True)
            hsum = scanp.tile([32, L], F16, tag=f"hsum{g}")
            nc.vector.tensor_add(hsum, h_st[0:32, W + 1:S + 1], hb_perm)
            negone = scanp.tile([32, 1], F32, tag=f"negone{g}")
            nc.vector.memset(negone[:, :], -1.0)
            exps = scanp.tile([32, L], F32, tag=f"exps{g}")
            s1 = scanp.tile([32, 1], F32, tag=f"s1{g}")
            # exp(0.5*hsum - 1) in [e^-2, 1]: stable without max-subtraction
            nc.scalar.activation(exps, hsum, AF.Exp, bias=negone[:, :],
                                 scale=0.5, accum_out=s1)
            ps_s = psb[0:2, 300:301]
            nc.tensor.matmul(ps_s, lhsT=sel_sb, rhs=s1, start=True, stop=True)
            r2 = scanp.tile([2, 1], F32, tag=f"r2{g}")
            nc.vector.reciprocal(r2, ps_s)
            ps_r = psb[:, 320:321]
            nc.tensor.matmul(ps_r, lhsT=selT_sb, rhs=r2, start=True, stop=True)
            att_r = scanp.tile([32, L], F16, tag=f"attr{g}")
            nc.vector.tensor_scalar_mul(att_r, exps, ps_r[:, 0:1])
            # flatten rows into token order: row r=(brel*16+k) -> offset r*L
            # (gpsimd queue: keeps the sync queue free for loads/stores)
            nc.gpsimd.dma_start(
                out=datt_g[g][0:1, :].rearrange("p (r s) -> p r s", r=32),
                in_=att_r[:, :])

        def emit_p5_block(g, i):
            tt = g * NP5 + i
            cols = slice(tt * P5CB, (tt + 1) * P5CB)
            # broadcast att across 128 partitions with a stride-0 SWDGE DMA
            # (gpsimd queue idle in p5; no PE/ACT involvement)
            pa = papool.tile([128, P5CB], F16, tag="pa")
            nc.gpsimd.dma_start(
                out=pa,
                in_=bass.AP(tensor=datt_g[g][:, :].tensor, offset=i * P5CB,
                            ap=[[0, 128], [1, P5CB]]))
            ob0 = opool.tile([128, P5CB], F16, tag="ob")
            nc.vector.tensor_mul(ob0, xT0[:, cols], pa)
            nc.sync.dma_start(out=outT[0:128, cols], in_=ob0)
            ob1 = opool.tile([128, P5CB], F16, tag="ob")
            nc.vector.tensor_mul(ob1, xT1[:, cols], pa)
            nc.scalar.dma_start(out=outT[128:256, cols], in_=ob1)
            ob2 = o2pool.tile([44, P5CB], F16, tag="ob2")
            nc.vector.tensor_mul(ob2, xT2[:, cols], pa[0:44, :])
            nc.scalar.dma_start(out=outT[256:300, cols], in_=ob2)

        # ---------------- pipeline schedule ----------------
        for i in range(NBG):
            emit_p1_block(0, i)
        # 2:1 interleave: scan iterations land early in each engine's
        # in-order stream; att's PE/ACT ops precede later blocks so the
        # engines reach them as soon as dependencies allow.
        emit_p1_block(1, 0)
        emit_scan_iter(0, 0)
        emit_scan_iter(0, 1)
        emit_p1_block(1, 1)
        emit_scan_iter(0, 2)
        emit_scan_iter(0, 3)
        emit_att(0)
        emit_p5_block(0, 0)
        emit_scan_iter(1, 0)
        emit_p5_block(0, 1)
        emit_scan_iter(1, 1)
        emit_p5_block(0, 2)
        emit_scan_iter(1, 2)
        emit_p5_block(0, 3)
        emit_scan_iter(1, 3)
        emit_att(1)
        for i in range(NP5):
            emit_p5_block(1, i)

    return nc


_NC = None


def _get_nc():
    global _NC
    if _NC is None:
        _NC = _build_nc()
        _NC.finalize()
    return _NC


def _prep_core_inputs(x, w_ih_f, w_hh_f, b_ih_f, b_hh_f,
                      w_ih_b, w_hh_b, b_ih_b, b_hh_b):
    """Build the per-core input maps."""
    w8T = np.zeros((E, 36), np.float32)
    b8 = np.zeros((36, 1), np.float32)
    whh = np.zeros((64, 4), np.float32)
    for d, (wi, wh, bi, bh) in enumerate(
            [(w_ih_f, w_hh_f, b_ih_f, b_hh_f),
             (w_ih_b, w_hh_b, b_ih_b, b_hh_b)]):
        for j, gp in enumerate(GATE_PERM):
            w8T[:, d * 32 + j] = wi[gp, :]
            b8[d * 32 + j, 0] = bi[gp] + bh[gp]
            # rows p = d*32 + brel*16 + k (per group): d = p // 32
            whh[d * 32:(d + 1) * 32, j] = wh[gp, 0]
    selG = np.zeros((32, 2), np.float32)
    for r in range(32):
        selG[r, r // 16] = 1.0
    selGT = np.ascontiguousarray(selG.T)
    permG = np.zeros((32, 32), NP16)
    for bb in range(2):
        for i in range(16):
            permG[bb * 16 + i, bb * 16 + 15 - i] = 1.0
    w8T16 = w8T.astype(NP16)

    maps = []
    for c in range(NCORES):
        xs = x[c * BL:(c + 1) * BL]                       # [4, T, E]
        xTc = np.ascontiguousarray(
            xs.transpose(2, 0, 1).reshape(E, TOK)).astype(NP16)
        maps.append({"xT": xTc, "w8T": w8T16, "b8": b8, "whh": whh,
                     "selG": selG, "selGT": selGT, "permG": permG})
    return maps


def _run(inputs, trace=False, tmpdir=None):
    nc = _get_nc()
    maps = _prep_core_inputs(**inputs)
    res = run_bass_kernel_spmd(nc, maps, list(range(NCORES)), trace=trace,
                               tmpdir=tmpdir)
    outs = []
    for c in range(NCORES):
        oT = res.results[c]["outT"].astype(np.float32)    # [E, TOK]
        outs.append(oT.reshape(E, BL, T).transpose(1, 2, 0))
    return np.concatenate(outs, axis=0), res


def kernel(**inputs):
    out, _ = _run(inputs, trace=False)
    return out
